# revision 17
# baseline (speedup 1.0000x reference)
# CondConv2d Trainium2 kernel (v3: 1-D Winograd F(2,3) along W, bf16).
#
# Math per sample n=(b,l):  out = conv2d(x*s, W) + fb   (see kernel.py)
#
# The 3x3 conv is computed as direct correlation along H (3 shifted row
# windows) x Winograd F(2,3) along W: 4*3 = 12 products per output pair
# vs 18 direct = 1.5x fewer tensor-engine cycles.  All conv matmuls bf16
# (hits the zero-overhead FWL weight path measured on hw); PSUM fp32.
# Numerically verified ~4e-3 scale-relative absmax vs the 2e-2 gate.
#
#   V0 = d0 - d2, V1 = d1 + d2, V2 = d2 - d1, V3 = d1 - d3
#     (d_c = col-padded scaled input col 2t+c-1, computed on GpSimd)
#   M[i][co,h,t] = sum_{kh,ci} w2[i,kh][co,ci] * V[i][ci,h+kh-1,t]  (PE)
#   out[:, h, 2t]   = M0 + M1 + M2 + fb     (ACT init + DVE increments)
#   out[:, h, 2t+1] = M1 - M2 - M3 + fb
#   w2[i] = sum_kw G[i,kw] * w[:,:,kh,kw],  G = F(2,3) weight transform
#
# PSUM: each M[i] owns a full 2KB bank ([128, 32, 16] fp32) because
# start_tensor_calc zeroes the entire bank; the output transform consumes
# M[i] incrementally (pp_m bufs=6).  Row padding handled by trimming the
# kh=0/kh=2 matmul row ranges, kh order [1,0,2] so the bank-starting
# matmul covers a full half.
#
# Schedule: software-pipelined emission — frame (b,l)'s pool/calib/scale/
# V-transform instructions are emitted one frame AHEAD of its conv, so the
# tensor queue never stalls on the DVE->ACT scale chain.  calib/gate are
# batched (l=0, l=1..3, b1 l=0..3) to keep tiny fp32 matmuls off the PE.

import numpy as np


def _install_axon_ntff_shim():
    # This container's `antenv` stub lacks `axon_hooks`, which
    # bass_utils imports unconditionally when trace=True under axon.
    import os
    import sys
    import types

    try:
        import antenv.axon_hooks  # noqa: F401

        return
    except Exception:
        pass
    try:
        import antenv
    except Exception:
        return
    mod = types.ModuleType("antenv.axon_hooks")
    mod._hook = None

    def set_axon_ntff_profile_hook(h):
        mod._hook = h

    def get_axon_ntff_profile_hook():
        return mod._hook

    mod.set_axon_ntff_profile_hook = set_axon_ntff_profile_hook
    mod.get_axon_ntff_profile_hook = get_axon_ntff_profile_hook
    sys.modules["antenv.axon_hooks"] = mod
    antenv.axon_hooks = mod
    try:
        from trn_agent_boot.trn_boot import _ntff_profile_via_ctypes

        so = "/opt/axon/libaxon_pjrt.so"
        if os.path.exists(so):
            mod._hook = _ntff_profile_via_ctypes(so)
    except Exception:
        pass


_install_axon_ntff_shim()

import ml_dtypes
import concourse.bass as bass
import concourse.tile as tile
from concourse import mybir
from concourse.bass_utils import run_bass_kernel_spmd

B, L, CIN, COUT, KS, H, W = 16, 4, 256, 256, 3, 32, 32
NCORES = 8
BS = B // NCORES      # batch samples per core
CC = CIN // 128       # ci chunks
OC = COUT // 128      # co chunks
WPAD = W + 2          # col-padded width
T = W // 2            # F(2,3) column tiles
FP32 = mybir.dt.float32
BF16 = mybir.dt.bfloat16
HHALF = 16
BF16_NP = ml_dtypes.bfloat16
ALU = mybir.AluOpType

_last_results = None


def _split_excess_waits(nc):
    # walrus in this toolchain encodes exactly one sem wait per engine
    # instruction, so split Tile's multi-wait instructions into standalone
    # EventSemaphore instructions on the same engine stream.
    n = 0
    f = nc.m.functions[0]
    for bb in f.blocks:
        insts = list(bb.instructions)
        out = []
        changed = False
        for inst in insts:
            si = inst.sync_info
            if si is not None:
                waits = list(si.on_wait)
                if len(waits) > 1:
                    for w in waits[:-1]:
                        n += 1
                        es = mybir.InstEventSemaphore(name=f"ES-SPLIT-{n}")
                        es.engine = inst.engine
                        es.sync_info = mybir.SyncInfo(on_wait=[w], on_update=[])
                        out.append(es)
                    si.on_wait = [waits[-1]]
                    inst.sync_info = si
                    changed = True
            out.append(inst)
        if changed:
            bb.instructions = out
    return n


def build_nc(split_waits=True):
    nc = bass.Bass()
    x_d = nc.dram_tensor("x", [BS, L, 128, CC, H, 2, T], BF16, kind="ExternalInput")
    w2_d = nc.dram_tensor("w2", [128, CC, 3, 4, COUT], BF16, kind="ExternalInput")
    tcw_d = nc.dram_tensor("tconv", [128, CC, 3, CIN], BF16, kind="ExternalInput")
    fcwr_d = nc.dram_tensor("fcr", [128, CC, 3, 128], BF16, kind="ExternalInput")
    bias_d = nc.dram_tensor("bias2", [128, OC], FP32, kind="ExternalInput")
    tb1_d = nc.dram_tensor("tb1", [128, CC], FP32, kind="ExternalInput")
    fcb1r_d = nc.dram_tensor("fcb1r", [128, 1], FP32, kind="ExternalInput")
    out_d = nc.dram_tensor("out", [BS, L, COUT, H, 2, T], FP32, kind="ExternalOutput")

    frames = [(b, l) for b in range(BS) for l in range(L)]

    with tile.TileContext(nc) as tc:
        with (
            tc.tile_pool(name="singles", bufs=1) as singles,
            tc.tile_pool(name="vpool", bufs=4) as vpool,
            tc.tile_pool(name="outp", bufs=4) as outp,
            tc.tile_pool(name="pp_m", bufs=7, space="PSUM") as pp_m,
            tc.tile_pool(name="pp_small", bufs=1, space="PSUM") as pp_small,
        ):
            # ---- persistent params, split so they land fast ----
            tcw_sb = singles.tile([128, CC, 3, CIN], BF16, tag="tcw")
            fcwr_sb = singles.tile([128, CC, 3, 128], BF16, tag="fcwr")
            bias_sb = singles.tile([128, OC], FP32, tag="bias")
            tb1_sb = singles.tile([128, CC], FP32, tag="tb1")
            fcb1r_sb = singles.tile([128, 1], FP32, tag="fcb1r")

            # ---- x staging: one tile + one DMA per frame ----
            x_r = {}
            for b, l in frames:
                x_r[(b, l)] = singles.tile(
                    [128, CC, H, 2, T], BF16, tag=f"xr{b}_{l}",
                    name=f"xr{b}_{l}",
                )

            def load_x(b, l, split=False):
                if split:
                    for ci in range(CC):
                        nc.sync.dma_start(out=x_r[(b, l)][:, ci],
                                          in_=x_d[b, l, :, ci])
                else:
                    nc.sync.dma_start(out=x_r[(b, l)][:], in_=x_d[b, l])

            load_x(0, 0, split=True)
            load_x(0, 1, split=True)
            nc.sync.dma_start(out=tcw_sb[:], in_=tcw_d[:])
            nc.sync.dma_start(out=fcwr_sb[:], in_=fcwr_d[:])
            nc.sync.dma_start(out=bias_sb[:], in_=bias_d[:])
            nc.sync.dma_start(out=tb1_sb[:], in_=tb1_d[:])
            nc.sync.dma_start(out=fcb1r_sb[:], in_=fcb1r_d[:])

            # transformed conv weights
            w2_sb = singles.tile([128, CC, 3, 4, COUT], BF16, tag="w2")
            for ci in range(CC):
                nc.sync.dma_start(out=w2_sb[:, ci], in_=w2_d[:, ci])

            for b, l in frames:
                if (b, l) not in ((0, 0), (0, 1)):
                    load_x(b, l)

            zcol_sb = singles.tile([128, H, 1], FP32, tag="zcol")
            nc.vector.memset(zcol_sb[:], 0.0)

            # ---- persistent per-sample state ----
            allxet = singles.tile([128, CC, BS, L + 2], FP32, tag="allxet")
            allxet_bf = singles.tile([128, CC, BS, L + 2], BF16, tag="allxet_bf")
            s_sb = singles.tile([128, CC, BS, L], FP32, tag="s")
            gbs_sb = singles.tile([128, BS, L], FP32, tag="gbs")
            fb_sb = singles.tile([128, BS, L, OC], FP32, tag="fb")

            # scaled x, even/odd column planes (bf16, contiguous).
            # XE[t]=x[2t] (t=0..15, col T zero-pad); XO[t]=x[2t-1]
            # (t=1..16, col 0 zero-pad).
            x_e, x_o = {}, {}
            for b, l in frames:
                for ci in range(CC):
                    x_e[(b, l, ci)] = singles.tile(
                        [128, H, T + 1], BF16, tag=f"xe{b}_{l}_{ci}",
                        name=f"xe{b}_{l}_{ci}",
                    )
                    x_o[(b, l, ci)] = singles.tile(
                        [128, H, T + 1], BF16, tag=f"xo{b}_{l}_{ci}",
                        name=f"xo{b}_{l}_{ci}",
                    )

            # ---------------- emission helpers ----------------
            def emit_pool(b, l):
                for ci in range(CC):
                    nc.vector.reduce_sum(
                        out=allxet[:, ci, b, 2 + l:3 + l],
                        in_=x_r[(b, l)][:, ci],
                        axis=mybir.AxisListType.XYZ,
                    )

            def emit_dups(b):
                for ci in range(CC):
                    nc.vector.tensor_copy(allxet[:, ci, b, 0:1],
                                          allxet[:, ci, b, 2:3])
                    nc.vector.tensor_copy(allxet[:, ci, b, 1:2],
                                          allxet[:, ci, b, 2:3])

            def emit_calib_gate(b, l0, n):
                # frames l0..l0+n-1; frame l uses allxet slots [l, l+1, l+2]
                # round the pooled window to bf16 so the calib/gate matmuls
                # hit the fast bf16 weight-load path
                nc.vector.tensor_copy(
                    allxet_bf[:, :, b, l0:l0 + n + 2], allxet[:, :, b, l0:l0 + n + 2]
                )
                for oc in range(OC):
                    pc = pp_small.tile([128, L], FP32, tag="pc",
                                       name=f"pc{b}_{l0}_{oc}")
                    mms = [(ci, k) for ci in range(CC) for k in range(3)]
                    for i, (ci, k) in enumerate(mms):
                        nc.tensor.matmul(
                            pc[:, 0:n],
                            lhsT=tcw_sb[:, ci, k, oc * 128:(oc + 1) * 128],
                            rhs=allxet_bf[:, ci, b, l0 + k:l0 + k + n],
                            start=(i == 0),
                            stop=(i == len(mms) - 1),
                        )
                    nc.vector.tensor_scalar_add(
                        s_sb[:, oc, b, l0:l0 + n], pc[:, 0:n],
                        tb1_sb[:, oc:oc + 1]
                    )
                pg = pp_small.tile([128, L], FP32, tag="pc",
                                   name=f"pg{b}_{l0}")
                mms = [(ci, k) for ci in range(CC) for k in range(3)]
                for i, (ci, k) in enumerate(mms):
                    nc.tensor.matmul(
                        pg[:, 0:n],
                        lhsT=fcwr_sb[:, ci, k, :],
                        rhs=allxet_bf[:, ci, b, l0 + k:l0 + k + n],
                        start=(i == 0),
                        stop=(i == len(mms) - 1),
                    )
                nc.vector.tensor_scalar_add(
                    gbs_sb[:, b, l0:l0 + n], pg[:, 0:n], fcb1r_sb[:, 0:1]
                )
                for l in range(l0, l0 + n):
                    nc.vector.tensor_scalar_mul(
                        fb_sb[:, b, l, :], bias_sb[:, :], gbs_sb[:, b, l:l + 1]
                    )

            def emit_pads_scale_v(b, l):
                for ci in range(CC):
                    nc.vector.tensor_copy(
                        x_e[(b, l, ci)][:, :, T:T + 1], zcol_sb[:])
                    nc.vector.tensor_copy(
                        x_o[(b, l, ci)][:, :, 0:1], zcol_sb[:])
                for ci in range(CC):
                    nc.scalar.mul(
                        x_e[(b, l, ci)][:, :, 0:T],
                        x_r[(b, l)][:, ci, :, 0, :],
                        s_sb[:, ci, b, l:l + 1],
                    )
                    nc.scalar.mul(
                        x_o[(b, l, ci)][:, :, 1:T + 1],
                        x_r[(b, l)][:, ci, :, 1, :],
                        s_sb[:, ci, b, l:l + 1],
                    )
                # V taps: d0=XO[t], d1=XE[t], d2=XO[t+1], d3=XE[t+1]
                V = {}
                for ci in range(CC):
                    V[ci] = vpool.tile([128, 4, H, T], BF16, tag="v",
                                       name=f"v{b}_{l}_{ci}")
                for i in range(4):
                    for ci in range(CC):
                        v = V[ci]
                        xe, xo = x_e[(b, l, ci)], x_o[(b, l, ci)]
                        e0, e1 = xe[:, :, 0:T], xe[:, :, 1:T + 1]
                        o0, o1 = xo[:, :, 0:T], xo[:, :, 1:T + 1]
                        if i == 0:
                            nc.gpsimd.tensor_sub(v[:, 0], o0, o1)
                        elif i == 1:
                            nc.gpsimd.tensor_add(v[:, 1], e0, o1)
                        elif i == 2:
                            nc.gpsimd.tensor_sub(v[:, 2], o1, e0)
                        else:
                            nc.gpsimd.tensor_sub(v[:, 3], e0, e1)
                return V

            def emit_conv(b, l, V):
                for oc in range(OC):
                    osb = outp.tile([128, H, 2, T], FP32, tag="osb",
                                    name=f"osb{b}_{l}_{oc}")
                    ev = osb[:, :, 0, :]
                    od = osb[:, :, 1, :]
                    fbap = fb_sb[:, b, l, oc:oc + 1]
                    ms = {}
                    for i in range(4):
                        m = pp_m.tile([128, H, T], FP32, tag="m",
                                      name=f"m{b}_{l}_{oc}_{i}")
                        ms[i] = m
                        # kh order [1,0,2]: the bank-starting matmul covers a
                        # full 16-row half; kh=0/2 rows trimmed (no row pads)
                        plan = []
                        for ci in range(CC):
                            for kh in (1, 0, 2):
                                dh = kh - 1
                                for half in range(2):
                                    h0 = half * HHALF
                                    hA = max(h0, -dh)
                                    hB = min(h0 + HHALF, H - dh)
                                    plan.append((ci, kh, dh, hA, hB))
                        for j, (ci, kh, dh, hA, hB) in enumerate(plan):
                            nc.tensor.matmul(
                                m[:, hA:hB, :],
                                lhsT=w2_sb[:, ci, kh, i,
                                           oc * 128:(oc + 1) * 128],
                                rhs=V[ci][:, i, hA + dh:hB + dh, :],
                                start=(j == 0),
                                stop=(j == len(plan) - 1),
                            )
                        # consume M[i] into the output halves
                        if i == 0:
                            nc.scalar.add(ev, m[:], fbap)
                        elif i == 1:
                            nc.vector.tensor_add(ev, ev, m[:])
                            nc.scalar.add(od, m[:], fbap)
                        elif i == 2:
                            nc.vector.tensor_add(ev, ev, m[:])
                            nc.vector.tensor_sub(od, od, m[:])
                        else:
                            nc.vector.tensor_sub(od, od, m[:])
                    nc.sync.dma_start(
                        out=out_d[b, l, oc * 128:(oc + 1) * 128],
                        in_=osb[:],
                    )

            # ---------------- pipelined emission ----------------
            prev = None  # (b, l, V) awaiting conv emission
            for idx, (b, l) in enumerate(frames):
                # pooling, two frames per iteration (b0 first, then b1)
                if idx < 2:
                    emit_pool(0, 2 * idx)
                    emit_pool(0, 2 * idx + 1)
                    if idx == 0:
                        emit_dups(0)
                elif idx < 4:
                    emit_pool(1, 2 * (idx - 2))
                    emit_pool(1, 2 * (idx - 2) + 1)
                    if idx == 2:
                        emit_dups(1)
                # batched calib/gate
                if (b, l) == (0, 0):
                    emit_calib_gate(0, 0, 1)
                elif (b, l) == (0, 1):
                    emit_calib_gate(0, 1, 3)
                elif (b, l) == (1, 0):
                    emit_calib_gate(1, 0, 4)
                V = emit_pads_scale_v(b, l)
                if prev is not None:
                    emit_conv(prev[0], prev[1], prev[2])
                prev = (b, l, V)
            emit_conv(prev[0], prev[1], prev[2])
    if split_waits:
        _split_excess_waits(nc)
    return nc


def host_pack(x, weight, bias, tconv_w, tconv_b, fc_w, fc_b):
    x = np.asarray(x, dtype=np.float32)
    weight = np.asarray(weight, dtype=np.float32)
    bias = np.asarray(bias, dtype=np.float32)
    tconv_w = np.asarray(tconv_w, dtype=np.float32)
    tconv_b = np.asarray(tconv_b, dtype=np.float32)
    fc_w = np.asarray(fc_w, dtype=np.float32)
    fc_b = np.asarray(fc_b, dtype=np.float32)

    # F(2,3) weight transform along kw: w2[a,o,i,kh] = sum_kw G[a,kw] w[o,i,kh,kw]
    G = np.array([[1, 0, 0], [0.5, 0.5, 0.5], [0.5, -0.5, 0.5], [0, 0, 1]],
                 np.float32)
    w2 = np.einsum("ak,oihk->aoih", G, weight)
    w2_host = np.ascontiguousarray(
        w2.transpose(2, 3, 0, 1)            # [CIN, 3, 4, COUT]
        .reshape(CC, 128, 3, 4, COUT)
        .transpose(1, 0, 2, 3, 4)           # [128, CC, 3, 4, COUT]
    ).astype(BF16_NP)

    tcw_host = np.ascontiguousarray(
        tconv_w.transpose(1, 2, 0).reshape(CC, 128, 3, CIN).transpose(1, 0, 2, 3)
        / (H * W)
    ).astype(BF16_NP)
    fcw_pack = fc_w[0].reshape(CC, 128, 3).transpose(1, 0, 2) / (H * W)
    fcwr_host = np.ascontiguousarray(
        np.broadcast_to(fcw_pack[:, :, :, None], (128, CC, 3, 128))
    ).astype(BF16_NP)
    bias_host = np.ascontiguousarray(bias.reshape(OC, 128).T)
    tb1_host = np.ascontiguousarray(tconv_b.reshape(CC, 128).T) + 1.0
    fcb1r_host = np.full((128, 1), float(fc_b[0]) + 1.0, np.float32)
    # pack channels as [128, CC] (one contiguous DMA per frame) and
    # deinterleave columns into even/odd planes for contiguous V taps
    x_bf16 = np.ascontiguousarray(
        x.reshape(B, L, CC, 128, H, T, 2).transpose(0, 1, 3, 2, 4, 6, 5)
    ).astype(BF16_NP)

    in_maps = []
    for core in range(NCORES):
        in_maps.append({
            "x": np.ascontiguousarray(x_bf16[core * BS:(core + 1) * BS]),
            "w2": w2_host,
            "tconv": tcw_host,
            "fcr": fcwr_host,
            "bias2": bias_host,
            "tb1": tb1_host,
            "fcb1r": fcb1r_host,
        })
    return in_maps


def unpack_out(raw):
    # raw: [BS, L, COUT, H, 2, T] parity-split device layout
    return (raw.reshape(BS * L, COUT, H, 2, T).transpose(0, 1, 2, 4, 3)
            .reshape(BS * L, COUT, H, W))


def kernel(x, weight, bias, tconv_w, tconv_b, fc_w, fc_b):
    global _last_results
    in_maps = host_pack(x, weight, bias, tconv_w, tconv_b, fc_w, fc_b)
    nc = build_nc()
    res = run_bass_kernel_spmd(nc, in_maps, core_ids=list(range(NCORES)))
    _last_results = res
    # device output is parity-split [.., H, 2, T]; re-interleave on host
    out = np.concatenate([unpack_out(r["out"]) for r in res.results], axis=0)
    return out
